# revision 3
# baseline (speedup 1.0000x reference)
"""Trainium2 Bass/Tile kernel for nn_Decoder (GRU decoder with teacher forcing).

Math (per reference):
  zx  = [enc_h_feat, z]                    (B, 1056)
  h0  = zx @ W_dh.T + b_dh                 (B, 128)
  a0  = last_obs @ W_vel.T + b_vel         (B, 2)
  rel = (sg - last_obs[:, :2]) / dt        (B, 2)
  a_t = a0 if t==0 else fut_traj[t-1,:,2:4]
  x_t = [zx, a_t, rel]  -> GRUCell(x_t, h) -> mu_t, std_t

Device strategy (8 cores, batch-sharded, 2048 rows/core):
  - Feature-on-partition, batch-on-free layout; free chunks of 512.
  - Everything bf16 except PSUM (fp32) and final outputs (fp32).
  - Setup: [gi_r|gi_z|gi_n|h0](512 rows) = W_big.T @ XT with K=1065
    host-packed rows [zxT; sgT; loT; ones].  rel, b_ih, b_dh AND b_hh_{r,z}
    are folded into W_big / its ones-row on the host.
  - a-contributions: a3 for ALL steps resident in SBUF ([98, 6*BC] tile,
    quarter q of the steps on partitions {32q, 32q+1}) -> zero per-step DMAs;
    small-K matmuls with explicit tile_position.
  - Per step (bf16 matmuls, fp32 PSUM):
      psum_rz  = Whh_{r,z} @ h + Ka_{r,z} @ a_t + I @ gi_{r,z}
      rz       = sigmoid(psum_rz)                      [ScalarE]
      q        = (psum_hn + b_hh_n) * r                [DVE stt]
      psum_gin = I@gi_n + Ka_n @ a_t + I@q
      n        = tanh(psum_gin)                        [ScalarE]
      d = h - n [DVE 2x], e = z*d [GPSIMD], h' = n + e [DVE 2x]
      head: scattered-column lhsT accumulates mu/std pre-acts for ALL steps
      into 4 persistent PSUM tiles.
  - End: mu/std finals read the head PSUM directly (Identity / Exp(0.5x)).
  - DMAs: host packs xt/weights into contiguous [128, X] blobs; ~10 large
    input DMAs, all on the SP (sync) queue; nothing on the Pool queue.
Host does only sharding/transposes/weight packing (a0 is a (B,6)@(6,2)
matmul on host, ~0.4 MFLOP, negligible).
"""

import numpy as np
import ml_dtypes

import concourse.bass as bass
import concourse.mybir as mybir
import concourse.tile as tile
from concourse import bacc
from concourse.bass_utils import run_bass_kernel_spmd

F32 = mybir.dt.float32
BF16 = mybir.dt.bfloat16
AF = mybir.ActivationFunctionType
OP = mybir.AluOpType

B, T, MLP, ZD, H, NS, NP = 16384, 24, 1024, 32, 128, 6, 2
NCORES = 8
BC = B // NCORES            # 2048 rows per core
F = 512                     # free-dim chunk
NF = BC // F                # 4 chunks
KIN = MLP + ZD + NP + NS + 1  # 1065 = zx(1056) + sg(2) + lo(6) + ones(1)
NKC = (KIN + 127) // 128    # 9 K-chunks (8x128 + 41)
DT_CONST = 0.4 * 12
TQ = T // 4                 # steps per a3 partition-quarter (6)

# head accumulator rows: [mu0 xT | mu1 xT | pad | std0 xT | std1 xT]
STD_OFF = ((2 * T + 31) // 32) * 32   # 64
M_HEAD = STD_OFF + 2 * T              # 112


def build_nc(debug=False):
    nc = bacc.Bacc("TRN2", target_bir_lowering=False, debug=debug)

    # ---- DRAM I/O (all bf16 except biases / outputs) ----
    # xt packed: K-chunk k occupies cols [k*BC, (k+1)*BC) on partitions 0..kc
    xt_d = nc.dram_tensor("xtp", [128, NKC * BC], BF16, kind="ExternalInput").ap()
    # wbig packed: block (k, m) at cols [(k*4+m)*128, ...+128)
    wb_d = nc.dram_tensor("wbp", [128, NKC * 4 * 128], BF16, kind="ExternalInput").ap()
    # misc weights packed: [whht(384) | wmsx(T*112) | ident(128)]
    WPCK = 3 * H + T * M_HEAD + H
    wp_d = nc.dram_tensor("wpk", [128, WPCK], BF16, kind="ExternalInput").ap()
    # a3: quarter q at partitions {32q, 32q+1}; cols (t%TQ)*BC + j
    a3_d = nc.dram_tensor("a3p", [98, TQ * BC], BF16, kind="ExternalInput").ap()
    # ka: same partition layout; cols g*128.. for gate g in (r, z, n)
    ka_d = nc.dram_tensor("kap", [98, 3 * H], BF16, kind="ExternalInput").ap()
    # biases fp32: col0 = b_hh_n (128 rows); col1 = bmu (48); col2 = 0.5*bstd
    bia_d = nc.dram_tensor("bia", [128, 3], F32, kind="ExternalInput").ap()
    omu_d = nc.dram_tensor("omu", [2 * T, BC], F32, kind="ExternalOutput").ap()
    ostd_d = nc.dram_tensor("ostd", [2 * T, BC], F32, kind="ExternalOutput").ap()

    with tile.TileContext(nc) as tc:
        with tc.tile_pool(name="persist", bufs=1) as pp:
            gi_r = pp.tile([H, BC], BF16)
            gi_z = pp.tile([H, BC], BF16)
            gi_n = pp.tile([H, BC], BF16)
            hA = pp.tile([H, BC], BF16)
            hB = pp.tile([H, BC], BF16)
            wpk = pp.tile([128, WPCK], BF16)
            a3 = pp.tile([98, TQ * BC], BF16)
            ka = pp.tile([98, 3 * H], BF16)
            bia = pp.tile([128, 3], F32)
            mu_sb = pp.tile([2 * T, BC], F32)
            std_sb = pp.tile([2 * T, BC], F32)

            whht = wpk[:, 0:3 * H]
            wmsx = wpk[:, 3 * H:3 * H + T * M_HEAD]
            ident = wpk[:, 3 * H + T * M_HEAD:WPCK]
            bhhn = bia[:, 0:1]
            bmu = bia[0:2 * T, 1:2]
            bstd = bia[0:2 * T, 2:3]

            # input DMAs: priority order on the single SP HWDGE queue
            nc.sync.dma_start(wpk[:], wp_d)
            nc.sync.dma_start(ka[:], ka_d)
            nc.sync.dma_start(bia[:], bia_d)

            gi_dst = [gi_r, gi_z, gi_n, hA]

            # ---- setup: [gi | h0] = W_big.T @ XT  (bf16) ----
            with tc.tile_pool(name="xtp", bufs=1) as xtp, \
                 tc.tile_pool(name="wbp", bufs=1) as wbp, \
                 tc.tile_pool(name="sps", bufs=4, space="PSUM") as sps:
                wb = wbp.tile([128, NKC * 4 * 128], BF16, name="wb", tag="wb")
                nc.sync.dma_start(wb[:], wb_d)
                xt = xtp.tile([128, NKC * BC], BF16, name="xt", tag="xt")
                # 5 k-group DMAs so early matmuls start before the tail loads
                for g, (k0, k1) in enumerate([(0, 2), (2, 4), (4, 6), (6, 8), (8, 9)]):
                    nc.sync.dma_start(xt[:, k0 * BC:k1 * BC],
                                      xt_d[:, k0 * BC:k1 * BC])
                nc.sync.dma_start(a3[:], a3_d)

                for m in range(4):
                    for c in range(NF):
                        ps = sps.tile([128, F], F32, name="setps", tag="setps")
                        for k in range(NKC):
                            kc = min(128, KIN - 128 * k)
                            nc.tensor.matmul(
                                ps[:],
                                wb[0:kc, (k * 4 + m) * 128:(k * 4 + m) * 128 + 128],
                                xt[0:kc, k * BC + c * F:k * BC + c * F + F],
                                start=(k == 0), stop=(k == NKC - 1),
                            )
                        nc.vector.tensor_copy(gi_dst[m][:, c * F:(c + 1) * F], ps[:])

            # ---- recurrence (software-pipelined issue order) ----
            # Per (t, c) block the PE issues: head(t-1, c), then the 10 loop
            # matmuls for (t, c).  The elementwise tail for (t, c) is issued
            # one block later (deferred) so no engine queue head-of-line
            # blocks on a cross-engine dependency.
            with tc.tile_pool(name="gp", bufs=4) as gp, \
                 tc.tile_pool(name="ps", bufs=4, space="PSUM") as psp, \
                 tc.tile_pool(name="phd", bufs=1, space="PSUM") as phd:
                psum_hd = [
                    phd.tile([M_HEAD, F], F32, name=f"pshd{c}", tag=f"pshd{c}")
                    for c in range(NF)
                ]
                pend = None   # deferred tail: (t, c, ps_hn, ps_gin, gr, gz)

                def flush(nxt_q):
                    nonlocal pend
                    if pend is None:
                        return
                    pt, pc, ps_gin, gr, gz, q = pend
                    pend = None
                    pcs = slice(pc * F, (pc + 1) * F)
                    ph = hA if pt % 2 == 0 else hB
                    pn = hB if pt % 2 == 0 else hA
                    npre = gp.tile([128, F], BF16, name="np", tag="np")
                    nc.vector.tensor_tensor(npre[:], ps_gin[:], q[:], op=OP.add)
                    if nxt_q is not None:
                        nxt_q()   # q(t, c) right after npre(prev) on DVE
                    nt = gp.tile([128, F], BF16, name="nt", tag="nt")
                    nc.scalar.activation(nt[:], npre[:], AF.Tanh)
                    d = gp.tile([128, F], BF16, name="d", tag="d")
                    nc.vector.tensor_tensor(d[:], ph[:, pcs], nt[:], op=OP.subtract)
                    e = gp.tile([128, F], BF16, name="e", tag="e")
                    nc.gpsimd.tensor_tensor(e[:], gz[:], d[:], op=OP.mult)
                    nc.vector.tensor_tensor(pn[:, pcs], nt[:], e[:], op=OP.add)

                for t in range(T):
                    hcur = hA if t % 2 == 0 else hB
                    hnxt = hB if t % 2 == 0 else hA
                    hprv = hnxt  # h produced at t-1 lives in the other buffer
                    ar = 32 * (t // TQ)          # a3 partition base for this step
                    ac = (t % TQ) * BC           # a3 col base
                    for c in range(NF):
                        cs = slice(c * F, (c + 1) * F)
                        hs = hcur[:, cs]
                        a_sl = a3[ar:ar + 2, ac + c * F:ac + c * F + F]
                        if t > 0:
                            # head for (t-1, c): reads hcur (h of step t-1)
                            nc.tensor.matmul(
                                psum_hd[c][:],
                                wmsx[:, (t - 1) * M_HEAD:t * M_HEAD],
                                hs,
                                start=(t - 1 == 0), stop=False,
                                skip_group_check=True,
                            )
                        ps_r = psp.tile([128, F], F32, name="psr", tag="ps")
                        nc.tensor.matmul(ps_r[:], whht[:, 0:H], hs,
                                         start=True, stop=False)
                        nc.tensor.matmul(ps_r[:], ka[ar:ar + 2, 0:H], a_sl,
                                         start=False, stop=False,
                                         tile_position=(ar, 0))
                        nc.tensor.matmul(ps_r[:], ident, gi_r[:, cs],
                                         start=False, stop=True)
                        ps_z = psp.tile([128, F], F32, name="psz", tag="ps")
                        nc.tensor.matmul(ps_z[:], whht[:, H:2 * H], hs,
                                         start=True, stop=False)
                        nc.tensor.matmul(ps_z[:], ka[ar:ar + 2, H:2 * H], a_sl,
                                         start=False, stop=False,
                                         tile_position=(ar, 0))
                        nc.tensor.matmul(ps_z[:], ident, gi_z[:, cs],
                                         start=False, stop=True)
                        ps_hn = psp.tile([128, F], F32, name="psh", tag="ps")
                        nc.tensor.matmul(ps_hn[:], whht[:, 2 * H:3 * H], hs,
                                         start=True, stop=True)
                        ps_gin = psp.tile([128, F], F32, name="psg", tag="ps")
                        nc.tensor.matmul(ps_gin[:], ident, gi_n[:, cs],
                                         start=True, stop=False)
                        nc.tensor.matmul(ps_gin[:], ka[ar:ar + 2, 2 * H:3 * H],
                                         a_sl, start=False, stop=True,
                                         tile_position=(ar, 0))
                        gr = gp.tile([128, F], BF16, name="gr", tag="gr")
                        nc.scalar.activation(gr[:], ps_r[:], AF.Sigmoid)
                        gz = gp.tile([128, F], BF16, name="gz", tag="gz")
                        nc.scalar.activation(gz[:], ps_z[:], AF.Sigmoid)
                        q = gp.tile([128, F], BF16, name="q", tag="q")

                        def issue_q(q=q, ps_hn=ps_hn, gr=gr):
                            nc.vector.scalar_tensor_tensor(
                                q[:], ps_hn[:], bhhn, gr[:],
                                op0=OP.add, op1=OP.mult,
                            )
                        if pend is None:
                            issue_q(None)
                            flush(None)
                        else:
                            flush(issue_q)
                        pend = (t, c, ps_gin, gr, gz, q)
                flush(None)
                # post-loop heads for t = T-1 (reads h of the last step)
                hlast = hB if (T - 1) % 2 == 0 else hA
                for c in range(NF):
                    nc.tensor.matmul(
                        psum_hd[c][:],
                        wmsx[:, (T - 1) * M_HEAD:T * M_HEAD],
                        hlast[:, c * F:(c + 1) * F],
                        start=False, stop=True,
                        skip_group_check=True,
                    )

                # ---- finals straight from head PSUM ----
                for c in range(NF):
                    cs = slice(c * F, (c + 1) * F)
                    nc.scalar.activation(mu_sb[:, cs], psum_hd[c][0:2 * T, :],
                                         AF.Identity, bias=bmu)
                    nc.scalar.activation(std_sb[:, cs],
                                         psum_hd[c][STD_OFF:STD_OFF + 2 * T, :],
                                         AF.Exp, bias=bstd, scale=0.5)
            nc.sync.dma_start(omu_d, mu_sb[:])
            nc.sync.dma_start(ostd_d, std_sb[:])

    nc.compile()
    return nc


_NC_CACHE = {}


def _get_nc(debug=False):
    if "nc" not in _NC_CACHE:
        _NC_CACHE["nc"] = build_nc(debug=debug)
    return _NC_CACHE["nc"]


def make_in_maps(last_obs_state, enc_h_feat, z, sg, fut_traj,
                 W_dh, b_dh, W_vel, b_vel, W_ih, b_ih, W_hh, b_hh,
                 W_mu, b_mu, W_std, b_std):
    f32 = np.float32
    bf = ml_dtypes.bfloat16

    # ---- weight packing (core-independent) ----
    # W_big: (KIN, 512) ; out cols = [gi_r, gi_z, gi_n, h0]
    wbig = np.zeros((KIN, 512), f32)
    wbig[0:1056, 0:384] = W_ih[:, 0:1056].T
    wbig[0:1056, 384:512] = W_dh.T
    # sg rows: rel = (sg - lo[:, :2])/dt feeds W_ih[:, 1058:1060]
    wbig[1056:1058, 0:384] = (W_ih[:, 1058:1060] / DT_CONST).T
    # lo rows (6): first two carry -W_rel/dt
    wbig[1058:1060, 0:384] = (-W_ih[:, 1058:1060] / DT_CONST).T
    # ones row: input-side biases; b_hh_{r,z} folded in here too
    wbig[1064, 0:384] = b_ih
    wbig[1064, 0:256] += b_hh[0:256]
    wbig[1064, 384:512] = b_dh
    # packed layout [128, NKC*4*128]: block (k, m) at cols (k*4+m)*128
    wbp = np.zeros((128, NKC * 4 * 128), f32)
    for k in range(NKC):
        kc = min(128, KIN - 128 * k)
        for m in range(4):
            wbp[0:kc, (k * 4 + m) * 128:(k * 4 + m) * 128 + 128] = \
                wbig[128 * k:128 * k + kc, 128 * m:128 * (m + 1)]
    wbp = wbp.astype(bf)

    # misc weights packed: [whht | wmsx | ident]
    whht = np.ascontiguousarray(W_hh.T)                     # (128, 384)
    wmsx = np.zeros((H, T, M_HEAD), f32)
    for t in range(T):
        wmsx[:, t, t] = W_mu[0]
        wmsx[:, t, T + t] = W_mu[1]
        wmsx[:, t, STD_OFF + t] = W_std[0]
        wmsx[:, t, STD_OFF + T + t] = W_std[1]
    wpk = np.concatenate(
        [whht, wmsx.reshape(H, T * M_HEAD), np.eye(H, dtype=f32)], axis=1
    ).astype(bf)

    # ka: Wa rows for each quarter at partitions {32q, 32q+1}
    kap = np.zeros((98, 3 * H), f32)
    for qq in range(4):
        kap[32 * qq:32 * qq + 2, 0:H] = W_ih[0:128, 1056:1058].T
        kap[32 * qq:32 * qq + 2, H:2 * H] = W_ih[128:256, 1056:1058].T
        kap[32 * qq:32 * qq + 2, 2 * H:3 * H] = W_ih[256:384, 1056:1058].T
    kap = kap.astype(bf)

    bia = np.zeros((128, 3), f32)
    bia[:, 0] = b_hh[256:384]
    bia[0:2 * T, 1] = np.repeat(b_mu, T)
    bia[0:2 * T, 2] = 0.5 * np.repeat(b_std, T)

    # host-side tiny matmul for a0 (0.4 MFLOP)
    a0 = last_obs_state @ W_vel.T + b_vel                    # (B, 2)

    in_maps = []
    for cidx in range(NCORES):
        sl = slice(cidx * BC, (cidx + 1) * BC)
        xt = np.empty((KIN, BC), f32)
        xt[0:MLP] = enc_h_feat[sl].T
        xt[MLP:1056] = z[sl].T
        xt[1056:1058] = sg[sl].T
        xt[1058:1064] = last_obs_state[sl].T
        xt[1064] = 1.0
        xtp = np.zeros((128, NKC * BC), f32)
        for k in range(NKC):
            kc = min(128, KIN - 128 * k)
            xtp[0:kc, k * BC:k * BC + BC] = xt[128 * k:128 * k + kc]
        xtp = xtp.astype(bf)
        # a3 packed: quarter q at partitions {32q, 32q+1}, cols (t%TQ)*BC+j
        a3 = np.zeros((98, TQ * BC), f32)
        for t in range(T):
            at = a0[sl] if t == 0 else fut_traj[t - 1, sl, 2:4]   # (BC, 2)
            qq, tq = t // TQ, t % TQ
            a3[32 * qq:32 * qq + 2, tq * BC:(tq + 1) * BC] = at.T
        a3 = a3.astype(bf)
        in_maps.append({
            "xtp": xtp,
            "wbp": wbp,
            "wpk": wpk,
            "a3p": a3,
            "kap": kap,
            "bia": bia,
        })
    return in_maps


def unpack_outputs(results):
    mus = np.empty((T, B, 2), np.float32)
    stds = np.empty((T, B, 2), np.float32)
    for c in range(NCORES):
        sl = slice(c * BC, (c + 1) * BC)
        omu = results[c]["omu"].reshape(2, T, BC)
        ostd = results[c]["ostd"].reshape(2, T, BC)
        mus[:, sl, 0] = omu[0]
        mus[:, sl, 1] = omu[1]
        stds[:, sl, 0] = ostd[0]
        stds[:, sl, 1] = ostd[1]
    return mus, stds


def kernel(last_obs_state, enc_h_feat, z, sg, fut_traj,
           W_dh, b_dh, W_vel, b_vel, W_ih, b_ih, W_hh, b_hh,
           W_mu, b_mu, W_std, b_std):
    args = dict(
        last_obs_state=np.asarray(last_obs_state, np.float32),
        enc_h_feat=np.asarray(enc_h_feat, np.float32),
        z=np.asarray(z, np.float32),
        sg=np.asarray(sg, np.float32),
        fut_traj=np.asarray(fut_traj, np.float32),
        W_dh=np.asarray(W_dh, np.float32), b_dh=np.asarray(b_dh, np.float32),
        W_vel=np.asarray(W_vel, np.float32), b_vel=np.asarray(b_vel, np.float32),
        W_ih=np.asarray(W_ih, np.float32), b_ih=np.asarray(b_ih, np.float32),
        W_hh=np.asarray(W_hh, np.float32), b_hh=np.asarray(b_hh, np.float32),
        W_mu=np.asarray(W_mu, np.float32), b_mu=np.asarray(b_mu, np.float32),
        W_std=np.asarray(W_std, np.float32), b_std=np.asarray(b_std, np.float32),
    )
    nc = _get_nc()
    in_maps = make_in_maps(**args)
    res = run_bass_kernel_spmd(nc, in_maps, core_ids=list(range(NCORES)))
    return unpack_outputs(res.results)


# revision 15
# speedup vs baseline: 1.5654x; 1.5654x over previous
"""Trainium2 Bass/Tile kernel for nn_Decoder (GRU decoder with teacher forcing).

Math (per reference):
  zx  = [enc_h_feat, z]                    (B, 1056)
  h0  = zx @ W_dh.T + b_dh                 (B, 128)
  a0  = last_obs @ W_vel.T + b_vel         (B, 2)
  rel = (sg - last_obs[:, :2]) / dt        (B, 2)
  a_t = a0 if t==0 else fut_traj[t-1,:,2:4]
  x_t = [zx, a_t, rel]  -> GRUCell(x_t, h) -> mu_t, std_t

Device strategy (8 cores, batch-sharded, 2048 rows/core):
  - Feature-on-partition, batch-on-free layout; free chunks of 512.
  - Everything bf16 except PSUM (fp32) and final outputs (fp32).
  - Setup: [gi_r|gi_z|gi_n|h0](512 rows) = W_big.T @ XT with K=1065
    host-packed rows [zxT; sgT; loT; ones].  rel, b_ih, b_dh AND b_hh_{r,z}
    are folded into W_big / its ones-row on the host.
  - a-contributions: a3 for ALL steps resident in SBUF ([98, 6*BC] tile,
    quarter q of the steps on partitions {32q, 32q+1}) -> zero per-step DMAs;
    small-K matmuls with explicit tile_position.
  - Per step (bf16 matmuls, fp32 PSUM):
      psum_rz  = Whh_{r,z} @ h + Ka_{r,z} @ a_t + I @ gi_{r,z}
      rz       = sigmoid(psum_rz)                      [ScalarE]
      q        = (psum_hn + b_hh_n) * r                [DVE stt]
      psum_gin = I@gi_n + Ka_n @ a_t + I@q
      n        = tanh(psum_gin)                        [ScalarE]
      d = h - n [DVE 2x], e = z*d [GPSIMD], h' = n + e [DVE 2x]
      head: scattered-column lhsT accumulates mu/std pre-acts for ALL steps
      into 4 persistent PSUM tiles.
  - End: mu/std finals read the head PSUM directly (Identity / Exp(0.5x)).
  - DMAs: host packs xt/weights into contiguous [128, X] blobs; ~10 large
    input DMAs, all on the SP (sync) queue; nothing on the Pool queue.
Host does only sharding/transposes/weight packing (a0 is a (B,6)@(6,2)
matmul on host, ~0.4 MFLOP, negligible).
"""

import numpy as np
import ml_dtypes

import concourse.bass as bass
import concourse.mybir as mybir
import concourse.tile as tile
from concourse import bacc
from concourse.bass_utils import run_bass_kernel_spmd

F32 = mybir.dt.float32
F32R = mybir.dt.float32r
BF16 = mybir.dt.bfloat16
FP8 = mybir.dt.float8e4
DR = mybir.MatmulPerfMode.DoubleRow
AF = mybir.ActivationFunctionType
OP = mybir.AluOpType

B, T, MLP, ZD, H, NS, NP = 16384, 24, 1024, 32, 128, 6, 2
NCORES = 8
BC = B // NCORES            # 2048 rows per core
F = 512                     # free-dim chunk
NF = BC // F                # 4 chunks
KIN = MLP + ZD + NP + NS + 1  # 1065 = zx(1056) + sg(2) + lo(6) + ones(1)
NKC = (KIN + 127) // 128    # 9 K-chunks (8x128 + 41)
DT_CONST = 0.4 * 12
TQ = T // 4                 # steps per a3 partition-quarter (6)

# head accumulator rows: [mu0 xT | mu1 xT | pad | std0 xT | std1 xT]
STD_OFF = ((2 * T + 31) // 32) * 32   # 64
M_HEAD = STD_OFF + 2 * T              # 112


def build_nc(debug=False):
    nc = bacc.Bacc("TRN2", target_bir_lowering=False, debug=debug)

    # ---- DRAM I/O (all bf16 except biases / outputs) ----
    # xt packed: K-chunk k occupies cols [k*BC, (k+1)*BC) on partitions 0..kc
    xt_d = nc.dram_tensor("xtp", [128, NKC * BC], BF16, kind="ExternalInput").ap()
    # wbig packed: block (k, m) at cols [(k*4+m)*128, ...+128)
    wb_d = nc.dram_tensor("wbp", [128, NKC * 4 * 128], BF16, kind="ExternalInput").ap()
    # misc weights packed: [whht(384) | wmsx(T*112) | ident(128)]
    WPCK = 3 * H + T * M_HEAD + H
    wp_d = nc.dram_tensor("wpk", [128, WPCK], F32, kind="ExternalInput").ap()
    # a3: quarter q at partitions {32q, 32q+1}; cols (t%TQ)*BC + j
    a3_d = nc.dram_tensor("a3p", [98, 2, TQ * BC], FP8, kind="ExternalInput").ap()
    # ka: same partition layout; cols g*128.. for gate g in (r, z, n)
    ka_d = nc.dram_tensor("kap", [98, 2, 3 * H], FP8, kind="ExternalInput").ap()
    # biases fp32: col0 = b_hh_n (128 rows); col1 = bmu (48); col2 = 0.5*bstd
    bia_d = nc.dram_tensor("bia", [128, 3], F32, kind="ExternalInput").ap()
    omu_d = nc.dram_tensor("omu", [2 * T, BC], F32, kind="ExternalOutput").ap()
    ostd_d = nc.dram_tensor("ostd", [2 * T, BC], F32, kind="ExternalOutput").ap()

    with tile.TileContext(nc) as tc:
        with tc.tile_pool(name="persist", bufs=1) as pp:
            gi_r = pp.tile([H, BC], F32R)
            gi_z = pp.tile([H, BC], F32R)
            gi_n = pp.tile([H, BC], F32R)
            hA = pp.tile([H, BC], F32R)
            hB = pp.tile([H, BC], F32R)
            wpk = pp.tile([128, WPCK], F32R)
            a3 = pp.tile([98, 2, TQ * BC], FP8)
            ka = pp.tile([98, 2, 3 * H], FP8)
            bia = pp.tile([128, 3], F32)
            mu_sb = pp.tile([2 * T, BC], F32)
            std_sb = pp.tile([2 * T, BC], F32)

            whht = wpk[:, 0:3 * H]
            wmsx = wpk[:, 3 * H:3 * H + T * M_HEAD]
            ident = wpk[:, 3 * H + T * M_HEAD:WPCK]
            bhhn = bia[:, 0:1]
            bmu = bia[0:2 * T, 1:2]
            bstd = bia[0:2 * T, 2:3]


            gi_dst = [gi_r, gi_z, gi_n, hA]

            # ---- setup: [gi | h0] = W_big.T @ XT  (bf16) ----
            with tc.tile_pool(name="xtp", bufs=1) as xtp, \
                 tc.tile_pool(name="wbp", bufs=1) as wbp, \
                 tc.tile_pool(name="sps", bufs=4, space="PSUM") as sps:
                wb = wbp.tile([128, NKC * 4 * 128], BF16, name="wb", tag="wb")
                xt = xtp.tile([128, NKC * BC], BF16, name="xt", tag="xt")
                # interleaved k-group DMAs: first matmuls start after ~2 groups
                for k0, k1 in [(0, 2), (2, 4), (4, 6), (6, 8), (8, 9)]:
                    nc.sync.dma_start(wb[:, k0 * 512:k1 * 512],
                                      wb_d[:, k0 * 512:k1 * 512])
                    nc.sync.dma_start(xt[:, k0 * BC:k1 * BC],
                                      xt_d[:, k0 * BC:k1 * BC])
                # loop-phase inputs (needed only after setup finishes)
                nc.sync.dma_start(wpk[:], wp_d.bitcast(F32R))
                nc.sync.dma_start(ka[:], ka_d)
                nc.sync.dma_start(bia[:], bia_d)
                nc.sync.dma_start(a3[:], a3_d)

                for m in range(4):
                    for c in range(NF):
                        ps = sps.tile([128, F], F32, name="setps", tag="setps")
                        for k in range(NKC):
                            kc = min(128, KIN - 128 * k)
                            nc.tensor.matmul(
                                ps[:],
                                wb[0:kc, (k * 4 + m) * 128:(k * 4 + m) * 128 + 128],
                                xt[0:kc, k * BC + c * F:k * BC + c * F + F],
                                start=(k == 0), stop=(k == NKC - 1),
                            )
                        nc.vector.tensor_copy(gi_dst[m][:, c * F:(c + 1) * F], ps[:])

            # ---- recurrence (software-pipelined issue order) ----
            # Per (t, c) block the PE issues: head(t-1, c), then the 10 loop
            # matmuls for (t, c).  The elementwise tail for (t, c) is issued
            # one block later (deferred) so no engine queue head-of-line
            # blocks on a cross-engine dependency.
            with tc.tile_pool(name="gp", bufs=4) as gp, \
                 tc.tile_pool(name="ps", bufs=4, space="PSUM") as psp, \
                 tc.tile_pool(name="phd", bufs=1, space="PSUM") as phd:
                psum_hd = [
                    phd.tile([M_HEAD, F], F32, name=f"pshd{c}", tag=f"pshd{c}")
                    for c in range(NF)
                ]
                from collections import deque
                pend = None   # (t, c, ps_gin, gr, gz, q) from the previous block
                hq = deque()  # (t, c, nt, e) awaiting the h' add, issued 3 blocks late

                def issue_hprime():
                    pt, pc, nt, e = hq.popleft()
                    pn = hB if pt % 2 == 0 else hA
                    pcs = slice(pc * F, (pc + 1) * F)
                    nc.vector.tensor_tensor(pn[:, pcs], nt[:], e[:], op=OP.add)

                def flush_npre():
                    # DVE: npre for the previous block, ahead of this block's q
                    if pend is None:
                        return None
                    pt, pc, ps_gin, gr, gz, q = pend
                    npre = gp.tile([128, F], F32, name="np", tag="np")
                    nc.vector.tensor_tensor(npre[:], ps_gin[:], q[:], op=OP.add)
                    return npre

                def flush_tail(npre):
                    # Act: tanh (queued after this block's sigmoids);
                    # Pool: d, e.  h' enqueues for a later block.
                    nonlocal pend
                    if pend is None:
                        return
                    pt, pc, ps_gin, gr, gz, q = pend
                    pend = None
                    pcs = slice(pc * F, (pc + 1) * F)
                    ph = hA if pt % 2 == 0 else hB
                    nt = gp.tile([128, F], F32, name="nt", tag="nt")
                    nc.scalar.activation(nt[:], npre[:], AF.Tanh)
                    d = gp.tile([128, F], F32, name="d", tag="d")
                    nc.gpsimd.tensor_tensor(d[:], ph[:, pcs], nt[:], op=OP.subtract)
                    e = gp.tile([128, F], F32, name="e", tag="e")
                    nc.gpsimd.tensor_tensor(e[:], gz[:], d[:], op=OP.mult)
                    hq.append((pt, pc, nt, e))

                for t in range(T):
                    hcur = hA if t % 2 == 0 else hB
                    hnxt = hB if t % 2 == 0 else hA
                    hprv = hnxt  # h produced at t-1 lives in the other buffer
                    ar = 32 * (t // TQ)          # a3 partition base for this step
                    ac = (t % TQ) * BC           # a3 col base
                    for c in range(NF):
                        cs = slice(c * F, (c + 1) * F)
                        hs = hcur[:, cs]
                        a_sl = a3[ar:ar + 1, :, ac + c * F:ac + c * F + F]
                        if t > 0:
                            # head for (t-1, c): reads hcur (h of step t-1)
                            nc.tensor.matmul(
                                psum_hd[c][:],
                                wmsx[:, (t - 1) * M_HEAD:t * M_HEAD],
                                hs,
                                start=(t - 1 == 0), stop=False,
                                skip_group_check=True,
                            )
                        ps_r = psp.tile([128, F], F32, name="psr", tag="ps")
                        nc.tensor.matmul(ps_r[:], whht[:, 0:H], hs,
                                         start=True, stop=False)
                        nc.tensor.matmul(ps_r[:], ka[ar:ar + 1, :, 0:H], a_sl,
                                         start=False, stop=False, perf_mode=DR,
                                         tile_position=(ar, 0))
                        nc.tensor.matmul(ps_r[:], ident, gi_r[:, cs],
                                         start=False, stop=True)
                        ps_z = psp.tile([128, F], F32, name="psz", tag="ps")
                        nc.tensor.matmul(ps_z[:], whht[:, H:2 * H], hs,
                                         start=True, stop=False)
                        nc.tensor.matmul(ps_z[:], ka[ar:ar + 1, :, H:2 * H], a_sl,
                                         start=False, stop=False, perf_mode=DR,
                                         tile_position=(ar, 0))
                        nc.tensor.matmul(ps_z[:], ident, gi_z[:, cs],
                                         start=False, stop=True)
                        ps_hn = psp.tile([128, F], F32, name="psh", tag="ps")
                        nc.tensor.matmul(ps_hn[:], whht[:, 2 * H:3 * H], hs,
                                         start=True, stop=True)
                        ps_gin = psp.tile([128, F], F32, name="psg", tag="ps")
                        nc.tensor.matmul(ps_gin[:], ident, gi_n[:, cs],
                                         start=True, stop=False)
                        nc.tensor.matmul(ps_gin[:], ka[ar:ar + 1, :, 2 * H:3 * H],
                                         a_sl, start=False, stop=True, perf_mode=DR,
                                         tile_position=(ar, 0))
                        if len(hq) >= 2:
                            issue_hprime()
                        npre_prev = flush_npre()
                        gr = gp.tile([128, F], F32, name="gr", tag="gr")
                        nc.scalar.activation(gr[:], ps_r[:], AF.Sigmoid)
                        gz = gp.tile([128, F], F32, name="gz", tag="gz")
                        nc.scalar.activation(gz[:], ps_z[:], AF.Sigmoid)
                        q = gp.tile([128, F], F32, name="q", tag="q")
                        nc.vector.scalar_tensor_tensor(
                            q[:], ps_hn[:], bhhn, gr[:],
                            op0=OP.add, op1=OP.mult,
                        )
                        flush_tail(npre_prev)
                        pend = (t, c, ps_gin, gr, gz, q)
                npre_prev = flush_npre()
                flush_tail(npre_prev)
                while hq:
                    issue_hprime()
                # post-loop heads for t = T-1 (reads h of the last step)
                hlast = hB if (T - 1) % 2 == 0 else hA
                for c in range(NF):
                    nc.tensor.matmul(
                        psum_hd[c][:],
                        wmsx[:, (T - 1) * M_HEAD:T * M_HEAD],
                        hlast[:, c * F:(c + 1) * F],
                        start=False, stop=True,
                        skip_group_check=True,
                    )

                # ---- finals straight from head PSUM ----
                for c in range(NF):
                    cs = slice(c * F, (c + 1) * F)
                    nc.scalar.activation(mu_sb[:, cs], psum_hd[c][0:2 * T, :],
                                         AF.Identity, bias=bmu)
                    nc.scalar.activation(std_sb[:, cs],
                                         psum_hd[c][STD_OFF:STD_OFF + 2 * T, :],
                                         AF.Exp, bias=bstd, scale=0.5)
            nc.sync.dma_start(omu_d, mu_sb[:])
            nc.sync.dma_start(ostd_d, std_sb[:])

    nc.compile()
    return nc


_NC_CACHE = {}


def _get_nc(debug=False):
    if "nc" not in _NC_CACHE:
        _NC_CACHE["nc"] = build_nc(debug=debug)
    return _NC_CACHE["nc"]


def make_in_maps(last_obs_state, enc_h_feat, z, sg, fut_traj,
                 W_dh, b_dh, W_vel, b_vel, W_ih, b_ih, W_hh, b_hh,
                 W_mu, b_mu, W_std, b_std):
    f32 = np.float32
    bf = ml_dtypes.bfloat16
    f8 = ml_dtypes.float8_e4m3

    # ---- weight packing (core-independent) ----
    # W_big: (KIN, 512) ; out cols = [gi_r, gi_z, gi_n, h0]
    wbig = np.zeros((KIN, 512), f32)
    wbig[0:1056, 0:384] = W_ih[:, 0:1056].T
    wbig[0:1056, 384:512] = W_dh.T
    # sg rows: rel = (sg - lo[:, :2])/dt feeds W_ih[:, 1058:1060]
    wbig[1056:1058, 0:384] = (W_ih[:, 1058:1060] / DT_CONST).T
    # lo rows (6): first two carry -W_rel/dt
    wbig[1058:1060, 0:384] = (-W_ih[:, 1058:1060] / DT_CONST).T
    # ones row: input-side biases; b_hh_{r,z} folded in here too
    wbig[1064, 0:384] = b_ih
    wbig[1064, 0:256] += b_hh[0:256]
    wbig[1064, 384:512] = b_dh
    # packed layout [128, NKC*4*128]: block (k, m) at cols (k*4+m)*128
    wbp = np.zeros((128, NKC * 4 * 128), f32)
    for k in range(NKC):
        kc = min(128, KIN - 128 * k)
        for m in range(4):
            wbp[0:kc, (k * 4 + m) * 128:(k * 4 + m) * 128 + 128] = \
                wbig[128 * k:128 * k + kc, 128 * m:128 * (m + 1)]
    wbp = wbp.astype(bf)
    

    # misc weights packed: [whht | wmsx | ident]
    whht = np.ascontiguousarray(W_hh.T)                     # (128, 384)
    wmsx = np.zeros((H, T, M_HEAD), f32)
    for t in range(T):
        wmsx[:, t, t] = W_mu[0]
        wmsx[:, t, T + t] = W_mu[1]
        wmsx[:, t, STD_OFF + t] = W_std[0]
        wmsx[:, t, STD_OFF + T + t] = W_std[1]
    wpk = np.concatenate(
        [whht, wmsx.reshape(H, T * M_HEAD), np.eye(H, dtype=f32)], axis=1
    ).astype(f32)

    # ka: Wa pairs for each quarter at partition 32q (DoubleRow fp8 layout)
    kap = np.zeros((98, 2, 3 * H), f32)
    for qq in range(4):
        for i in range(2):
            kap[32 * qq, i, 0:H] = W_ih[0:128, 1056 + i]
            kap[32 * qq, i, H:2 * H] = W_ih[128:256, 1056 + i]
            kap[32 * qq, i, 2 * H:3 * H] = W_ih[256:384, 1056 + i]
    kap = kap.astype(f8)

    bia = np.zeros((128, 3), f32)
    bia[:, 0] = b_hh[256:384]
    bia[0:2 * T, 1] = np.repeat(b_mu, T)
    bia[0:2 * T, 2] = 0.5 * np.repeat(b_std, T)

    # host-side tiny matmul for a0 (0.4 MFLOP)
    a0 = last_obs_state @ W_vel.T + b_vel                    # (B, 2)

    in_maps = []
    for cidx in range(NCORES):
        sl = slice(cidx * BC, (cidx + 1) * BC)
        xt = np.empty((KIN, BC), f32)
        xt[0:MLP] = enc_h_feat[sl].T
        xt[MLP:1056] = z[sl].T
        xt[1056:1058] = sg[sl].T
        xt[1058:1064] = last_obs_state[sl].T
        xt[1064] = 1.0
        xtp = np.zeros((128, NKC * BC), f32)
        for k in range(NKC):
            kc = min(128, KIN - 128 * k)
            xtp[0:kc, k * BC:k * BC + BC] = xt[128 * k:128 * k + kc]
        xtp = xtp.astype(bf)
        
        # a3 packed (DoubleRow fp8): quarter q at partition 32q, pair dim = a0/a1
        a3 = np.zeros((98, 2, TQ * BC), f32)
        for t in range(T):
            at = a0[sl] if t == 0 else fut_traj[t - 1, sl, 2:4]   # (BC, 2)
            qq, tq = t // TQ, t % TQ
            a3[32 * qq, :, tq * BC:(tq + 1) * BC] = at.T
        a3 = a3.astype(f8)
        in_maps.append({
            "xtp": xtp,
            "wbp": wbp,
            "wpk": wpk,
            "a3p": a3,
            "kap": kap,
            "bia": bia,
        })
    return in_maps


def unpack_outputs(results):
    mus = np.empty((T, B, 2), np.float32)
    stds = np.empty((T, B, 2), np.float32)
    for c in range(NCORES):
        sl = slice(c * BC, (c + 1) * BC)
        omu = results[c]["omu"].reshape(2, T, BC)
        ostd = results[c]["ostd"].reshape(2, T, BC)
        mus[:, sl, 0] = omu[0]
        mus[:, sl, 1] = omu[1]
        stds[:, sl, 0] = ostd[0]
        stds[:, sl, 1] = ostd[1]
    return mus, stds


def kernel(last_obs_state, enc_h_feat, z, sg, fut_traj,
           W_dh, b_dh, W_vel, b_vel, W_ih, b_ih, W_hh, b_hh,
           W_mu, b_mu, W_std, b_std):
    args = dict(
        last_obs_state=np.asarray(last_obs_state, np.float32),
        enc_h_feat=np.asarray(enc_h_feat, np.float32),
        z=np.asarray(z, np.float32),
        sg=np.asarray(sg, np.float32),
        fut_traj=np.asarray(fut_traj, np.float32),
        W_dh=np.asarray(W_dh, np.float32), b_dh=np.asarray(b_dh, np.float32),
        W_vel=np.asarray(W_vel, np.float32), b_vel=np.asarray(b_vel, np.float32),
        W_ih=np.asarray(W_ih, np.float32), b_ih=np.asarray(b_ih, np.float32),
        W_hh=np.asarray(W_hh, np.float32), b_hh=np.asarray(b_hh, np.float32),
        W_mu=np.asarray(W_mu, np.float32), b_mu=np.asarray(b_mu, np.float32),
        W_std=np.asarray(W_std, np.float32), b_std=np.asarray(b_std, np.float32),
    )
    nc = _get_nc()
    in_maps = make_in_maps(**args)
    res = run_bass_kernel_spmd(nc, in_maps, core_ids=list(range(NCORES)))
    return unpack_outputs(res.results)


# revision 39
# speedup vs baseline: 1.6020x; 1.0234x over previous
"""Trainium2 Bass/Tile kernel for nn_Decoder (GRU decoder with teacher forcing).

Math (per reference):
  zx  = [enc_h_feat, z]                    (B, 1056)
  h0  = zx @ W_dh.T + b_dh                 (B, 128)
  a0  = last_obs @ W_vel.T + b_vel         (B, 2)
  rel = (sg - last_obs[:, :2]) / dt        (B, 2)
  a_t = a0 if t==0 else fut_traj[t-1,:,2:4]
  x_t = [zx, a_t, rel]  -> GRUCell(x_t, h) -> mu_t, std_t

Device strategy (8 cores, batch-sharded, 2048 rows/core):
  - Feature-on-partition, batch-on-free layout; free chunks of 512.
  - Everything bf16 except PSUM (fp32) and final outputs (fp32).
  - Setup: [gi_r|gi_z|gi_n|h0](512 rows) = W_big.T @ XT with K=1065
    host-packed rows [zxT; sgT; loT; ones].  rel, b_ih, b_dh AND b_hh_{r,z}
    are folded into W_big / its ones-row on the host.
  - a-contributions: a3 for ALL steps resident in SBUF ([98, 6*BC] tile,
    quarter q of the steps on partitions {32q, 32q+1}) -> zero per-step DMAs;
    small-K matmuls with explicit tile_position.
  - Per step (bf16 matmuls, fp32 PSUM):
      psum_rz  = Whh_{r,z} @ h + Ka_{r,z} @ a_t + I @ gi_{r,z}
      rz       = sigmoid(psum_rz)                      [ScalarE]
      q        = (psum_hn + b_hh_n) * r                [DVE stt]
      psum_gin = I@gi_n + Ka_n @ a_t + I@q
      n        = tanh(psum_gin)                        [ScalarE]
      d = h - n [DVE 2x], e = z*d [GPSIMD], h' = n + e [DVE 2x]
      head: scattered-column lhsT accumulates mu/std pre-acts for ALL steps
      into 4 persistent PSUM tiles.
  - End: mu/std finals read the head PSUM directly (Identity / Exp(0.5x)).
  - DMAs: host packs xt/weights into contiguous [128, X] blobs; ~10 large
    input DMAs, all on the SP (sync) queue; nothing on the Pool queue.
Host does only sharding/transposes/weight packing (a0 is a (B,6)@(6,2)
matmul on host, ~0.4 MFLOP, negligible).
"""

import numpy as np
import ml_dtypes

import concourse.bass as bass
import concourse.mybir as mybir
import concourse.tile as tile
from concourse import bacc
from concourse.bass_utils import run_bass_kernel_spmd

F32 = mybir.dt.float32
F32R = mybir.dt.float32r
BF16 = mybir.dt.bfloat16
FP8 = mybir.dt.float8e4
DR = mybir.MatmulPerfMode.DoubleRow
AF = mybir.ActivationFunctionType
OP = mybir.AluOpType

B, T, MLP, ZD, H, NS, NP = 16384, 24, 1024, 32, 128, 6, 2
NCORES = 8
BC = B // NCORES            # 2048 rows per core
F = 512                     # free-dim chunk
NF = BC // F                # 4 chunks
KIN = MLP + ZD + NP + NS + 1  # 1065 = zx(1056) + sg(2) + lo(6) + ones(1)
NKC = (KIN + 127) // 128    # 9 K-chunks (8x128 + 41)
DT_CONST = 0.4 * 12
TQ = T // 4                 # steps per a3 partition-quarter (6)

# head accumulator rows: [mu0 xT | mu1 xT | pad | std0 xT | std1 xT]
STD_OFF = ((2 * T + 31) // 32) * 32   # 64
M_HEAD = STD_OFF + 2 * T              # 112


def build_nc(debug=False):
    nc = bacc.Bacc("TRN2", target_bir_lowering=False, debug=debug)

    # ---- DRAM I/O (all bf16 except biases / outputs) ----
    # xt packed: K-chunk k occupies cols [k*BC, (k+1)*BC) on partitions 0..kc
    xt_d = nc.dram_tensor("xtp", [128, NKC * BC], BF16, kind="ExternalInput").ap()
    # wbig packed: block (k, m) at cols [(k*4+m)*128, ...+128)
    wb_d = nc.dram_tensor("wbp", [128, NKC * 4 * 128], BF16, kind="ExternalInput").ap()
    # misc weights packed: [whht(384) | wmsx(T*112) | ident(128)]
    WPCK = 3 * H + T * M_HEAD + H
    wp_d = nc.dram_tensor("wpk", [128, WPCK], F32, kind="ExternalInput").ap()
    # a3: quarter q at partitions {32q, 32q+1}; cols (t%TQ)*BC + j
    a3_d = nc.dram_tensor("a3p", [98, 2, TQ * BC], FP8, kind="ExternalInput").ap()
    # ka: same partition layout; cols g*128.. for gate g in (r, z, n)
    ka_d = nc.dram_tensor("kap", [98, 2, 3 * H], FP8, kind="ExternalInput").ap()
    # biases fp32: col0 = b_hh_n (128 rows); col1 = bmu (48); col2 = 0.5*bstd
    bia_d = nc.dram_tensor("bia", [128, 3], F32, kind="ExternalInput").ap()
    omu_d = nc.dram_tensor("omu", [2 * T, BC], F32, kind="ExternalOutput").ap()
    ostd_d = nc.dram_tensor("ostd", [2 * T, BC], F32, kind="ExternalOutput").ap()

    with tile.TileContext(nc) as tc:
        with tc.tile_pool(name="persist", bufs=1) as pp:
            gi_r = pp.tile([H, BC], F32R)
            gi_z = pp.tile([H, BC], F32R)
            gi_n = pp.tile([H, BC], F32R)
            hA = pp.tile([H, BC], F32R)
            hB = pp.tile([H, BC], F32R)
            wpk = pp.tile([128, WPCK], F32R)
            a3 = pp.tile([98, 2, TQ * BC], FP8)
            ka = pp.tile([98, 2, 3 * H], FP8)
            bia = pp.tile([128, 3], F32)
            mu_sb = pp.tile([2 * T, BC], F32)
            std_sb = pp.tile([2 * T, BC], F32)

            whht = wpk[:, 0:3 * H]
            wmsx = wpk[:, 3 * H:3 * H + T * M_HEAD]
            ident = wpk[:, 3 * H + T * M_HEAD:WPCK]
            bhhn = bia[:, 0:1]
            bmu = bia[0:2 * T, 1:2]
            bstd = bia[0:2 * T, 2:3]


            gi_dst = [gi_r, gi_z, gi_n, hA]

            # ---- setup: [gi | h0] = W_big.T @ XT  (bf16) ----
            with tc.tile_pool(name="xtp", bufs=1) as xtp, \
                 tc.tile_pool(name="wbp", bufs=1) as wbp, \
                 tc.tile_pool(name="sps", bufs=8, space="PSUM") as sps:
                wb = wbp.tile([128, NKC * 4 * 128], BF16, name="wb", tag="wb")
                xt = xtp.tile([128, NKC * BC], BF16, name="xt", tag="xt")
                # interleaved per-k DMAs: first matmuls start after one pair
                for k in range(NKC):
                    nc.sync.dma_start(wb[:, k * 512:(k + 1) * 512],
                                      wb_d[:, k * 512:(k + 1) * 512])
                    nc.sync.dma_start(xt[:, k * BC:(k + 1) * BC],
                                      xt_d[:, k * BC:(k + 1) * BC])
                # loop-phase inputs (needed only after setup finishes)
                nc.sync.dma_start(wpk[:], wp_d.bitcast(F32R))
                nc.sync.dma_start(ka[:], ka_d)
                nc.sync.dma_start(bia[:], bia_d)
                nc.sync.dma_start(a3[:], a3_d)

                for m in range(4):
                    for c in range(NF):
                        ps = sps.tile([128, F], F32, name="setps", tag="setps")
                        for k in range(NKC):
                            kc = min(128, KIN - 128 * k)
                            nc.tensor.matmul(
                                ps[:],
                                wb[0:kc, (k * 4 + m) * 128:(k * 4 + m) * 128 + 128],
                                xt[0:kc, k * BC + c * F:k * BC + c * F + F],
                                start=(k == 0), stop=(k == NKC - 1),
                            )
                        nc.vector.tensor_copy(gi_dst[m][:, c * F:(c + 1) * F], ps[:])

            # ---- recurrence (software-pipelined issue order) ----
            # Per (t, c) block the PE issues: head(t-1, c), then the 10 loop
            # matmuls for (t, c).  The elementwise tail for (t, c) is issued
            # one block later (deferred) so no engine queue head-of-line
            # blocks on a cross-engine dependency.
            with tc.tile_pool(name="gp", bufs=4) as gp, \
                 tc.tile_pool(name="ps", bufs=4, space="PSUM") as psp, \
                 tc.tile_pool(name="phd", bufs=1, space="PSUM") as phd:
                psum_hd = [
                    phd.tile([M_HEAD, F], F32, name=f"pshd{c}", tag=f"pshd{c}")
                    for c in range(NF)
                ]
                from collections import deque
                pend = None   # (t, c, ps_gin, gr, gz, q) from the previous block
                hq = deque()  # (t, c, nt, e) awaiting the h' add, issued 3 blocks late

                def issue_hprime():
                    pt, pc, nt, e = hq.popleft()
                    pn = hB if pt % 2 == 0 else hA
                    pcs = slice(pc * F, (pc + 1) * F)
                    nc.vector.tensor_tensor(pn[:, pcs], nt[:], e[:], op=OP.add)

                def flush_npre():
                    # DVE: npre for the previous block, ahead of this block's q
                    if pend is None:
                        return None
                    pt, pc, ps_gin, gr, gz, q = pend
                    npre = gp.tile([128, F], F32, name="np", tag="np")
                    nc.vector.tensor_tensor(npre[:], ps_gin[:], q[:], op=OP.add)
                    return npre

                def flush_tail(npre):
                    # Act: tanh (queued after this block's sigmoids);
                    # Pool: d, e.  h' enqueues for a later block.
                    nonlocal pend
                    if pend is None:
                        return
                    pt, pc, ps_gin, gr, gz, q = pend
                    pend = None
                    pcs = slice(pc * F, (pc + 1) * F)
                    ph = hA if pt % 2 == 0 else hB
                    nt = gp.tile([128, F], F32, name="nt", tag="nt")
                    nc.scalar.activation(nt[:], npre[:], AF.Tanh)
                    d = gp.tile([128, F], F32, name="d", tag="d")
                    nc.gpsimd.tensor_tensor(d[:], ph[:, pcs], nt[:], op=OP.subtract)
                    e = gp.tile([128, F], F32, name="e", tag="e")
                    nc.gpsimd.tensor_tensor(e[:], gz[:], d[:], op=OP.mult)
                    hq.append((pt, pc, nt, e))

                for t in range(T):
                    hcur = hA if t % 2 == 0 else hB
                    hnxt = hB if t % 2 == 0 else hA
                    hprv = hnxt  # h produced at t-1 lives in the other buffer
                    ar = 32 * (t // TQ)          # a3 partition base for this step
                    ac = (t % TQ) * BC           # a3 col base
                    for c in range(NF):
                        cs = slice(c * F, (c + 1) * F)
                        hs = hcur[:, cs]
                        a_sl = a3[ar:ar + 1, :, ac + c * F:ac + c * F + F]
                        if t > 0:
                            # head for (t-1, c): reads hcur (h of step t-1)
                            nc.tensor.matmul(
                                psum_hd[c][:],
                                wmsx[:, (t - 1) * M_HEAD:t * M_HEAD],
                                hs,
                                start=(t - 1 == 0), stop=False,
                                skip_group_check=True,
                            )
                        ps_r = psp.tile([128, F], F32, name="psr", tag="ps")
                        nc.tensor.matmul(ps_r[:], whht[:, 0:H], hs,
                                         start=True, stop=False)
                        nc.tensor.matmul(ps_r[:], ka[ar:ar + 1, :, 0:H], a_sl,
                                         start=False, stop=False, perf_mode=DR,
                                         tile_position=(ar, 0))
                        nc.tensor.matmul(ps_r[:], ident, gi_r[:, cs],
                                         start=False, stop=True)
                        ps_z = psp.tile([128, F], F32, name="psz", tag="ps")
                        nc.tensor.matmul(ps_z[:], whht[:, H:2 * H], hs,
                                         start=True, stop=False)
                        nc.tensor.matmul(ps_z[:], ka[ar:ar + 1, :, H:2 * H], a_sl,
                                         start=False, stop=False, perf_mode=DR,
                                         tile_position=(ar, 0))
                        nc.tensor.matmul(ps_z[:], ident, gi_z[:, cs],
                                         start=False, stop=True)
                        ps_hn = psp.tile([128, F], F32, name="psh", tag="ps")
                        nc.tensor.matmul(ps_hn[:], whht[:, 2 * H:3 * H], hs,
                                         start=True, stop=True)
                        ps_gin = psp.tile([128, F], F32, name="psg", tag="ps")
                        nc.tensor.matmul(ps_gin[:], ident, gi_n[:, cs],
                                         start=True, stop=False)
                        nc.tensor.matmul(ps_gin[:], ka[ar:ar + 1, :, 2 * H:3 * H],
                                         a_sl, start=False, stop=True, perf_mode=DR,
                                         tile_position=(ar, 0))
                        if len(hq) >= 2:
                            issue_hprime()
                        npre_prev = flush_npre()
                        gr = gp.tile([128, F], F32, name="gr", tag="gr")
                        nc.scalar.activation(gr[:], ps_r[:], AF.Sigmoid)
                        gz = gp.tile([128, F], F32, name="gz", tag="gz")
                        nc.scalar.activation(gz[:], ps_z[:], AF.Sigmoid)
                        q = gp.tile([128, F], F32, name="q", tag="q")
                        nc.vector.scalar_tensor_tensor(
                            q[:], ps_hn[:], bhhn, gr[:],
                            op0=OP.add, op1=OP.mult,
                        )
                        flush_tail(npre_prev)
                        pend = (t, c, ps_gin, gr, gz, q)
                npre_prev = flush_npre()
                flush_tail(npre_prev)
                while hq:
                    issue_hprime()
                # post-loop heads for t = T-1 (reads h of the last step)
                hlast = hB if (T - 1) % 2 == 0 else hA
                for c in range(NF):
                    nc.tensor.matmul(
                        psum_hd[c][:],
                        wmsx[:, (T - 1) * M_HEAD:T * M_HEAD],
                        hlast[:, c * F:(c + 1) * F],
                        start=False, stop=True,
                        skip_group_check=True,
                    )

                # ---- finals straight from head PSUM ----
                for c in range(NF):
                    cs = slice(c * F, (c + 1) * F)
                    nc.scalar.activation(mu_sb[:, cs], psum_hd[c][0:2 * T, :],
                                         AF.Identity, bias=bmu)
                    nc.scalar.activation(std_sb[:, cs],
                                         psum_hd[c][STD_OFF:STD_OFF + 2 * T, :],
                                         AF.Exp, bias=bstd, scale=0.5)
            nc.sync.dma_start(omu_d, mu_sb[:])
            nc.sync.dma_start(ostd_d, std_sb[:])

    nc.compile()
    return nc


_NC_CACHE = {}


def _get_nc(debug=False):
    if "nc" not in _NC_CACHE:
        _NC_CACHE["nc"] = build_nc(debug=debug)
    return _NC_CACHE["nc"]


def make_in_maps(last_obs_state, enc_h_feat, z, sg, fut_traj,
                 W_dh, b_dh, W_vel, b_vel, W_ih, b_ih, W_hh, b_hh,
                 W_mu, b_mu, W_std, b_std):
    f32 = np.float32
    bf = ml_dtypes.bfloat16
    f8 = ml_dtypes.float8_e4m3

    # ---- weight packing (core-independent) ----
    # W_big: (KIN, 512) ; out cols = [gi_r, gi_z, gi_n, h0]
    wbig = np.zeros((KIN, 512), f32)
    wbig[0:1056, 0:384] = W_ih[:, 0:1056].T
    wbig[0:1056, 384:512] = W_dh.T
    # sg rows: rel = (sg - lo[:, :2])/dt feeds W_ih[:, 1058:1060]
    wbig[1056:1058, 0:384] = (W_ih[:, 1058:1060] / DT_CONST).T
    # lo rows (6): first two carry -W_rel/dt
    wbig[1058:1060, 0:384] = (-W_ih[:, 1058:1060] / DT_CONST).T
    # ones row: input-side biases; b_hh_{r,z} folded in here too
    wbig[1064, 0:384] = b_ih
    wbig[1064, 0:256] += b_hh[0:256]
    wbig[1064, 384:512] = b_dh
    # packed layout [128, NKC*4*128]: block (k, m) at cols (k*4+m)*128
    wbp = np.zeros((128, NKC * 4 * 128), f32)
    for k in range(NKC):
        kc = min(128, KIN - 128 * k)
        for m in range(4):
            wbp[0:kc, (k * 4 + m) * 128:(k * 4 + m) * 128 + 128] = \
                wbig[128 * k:128 * k + kc, 128 * m:128 * (m + 1)]
    wbp = wbp.astype(bf)
    

    # misc weights packed: [whht | wmsx | ident]
    whht = np.ascontiguousarray(W_hh.T)                     # (128, 384)
    wmsx = np.zeros((H, T, M_HEAD), f32)
    for t in range(T):
        wmsx[:, t, t] = W_mu[0]
        wmsx[:, t, T + t] = W_mu[1]
        wmsx[:, t, STD_OFF + t] = W_std[0]
        wmsx[:, t, STD_OFF + T + t] = W_std[1]
    wpk = np.concatenate(
        [whht, wmsx.reshape(H, T * M_HEAD), np.eye(H, dtype=f32)], axis=1
    ).astype(f32)

    # ka: Wa pairs for each quarter at partition 32q (DoubleRow fp8 layout)
    kap = np.zeros((98, 2, 3 * H), f32)
    for qq in range(4):
        for i in range(2):
            kap[32 * qq, i, 0:H] = W_ih[0:128, 1056 + i]
            kap[32 * qq, i, H:2 * H] = W_ih[128:256, 1056 + i]
            kap[32 * qq, i, 2 * H:3 * H] = W_ih[256:384, 1056 + i]
    kap = kap.astype(f8)

    bia = np.zeros((128, 3), f32)
    bia[:, 0] = b_hh[256:384]
    bia[0:2 * T, 1] = np.repeat(b_mu, T)
    bia[0:2 * T, 2] = 0.5 * np.repeat(b_std, T)

    # host-side tiny matmul for a0 (0.4 MFLOP)
    a0 = last_obs_state @ W_vel.T + b_vel                    # (B, 2)

    in_maps = []
    for cidx in range(NCORES):
        sl = slice(cidx * BC, (cidx + 1) * BC)
        xt = np.empty((KIN, BC), f32)
        xt[0:MLP] = enc_h_feat[sl].T
        xt[MLP:1056] = z[sl].T
        xt[1056:1058] = sg[sl].T
        xt[1058:1064] = last_obs_state[sl].T
        xt[1064] = 1.0
        xtp = np.zeros((128, NKC * BC), f32)
        for k in range(NKC):
            kc = min(128, KIN - 128 * k)
            xtp[0:kc, k * BC:k * BC + BC] = xt[128 * k:128 * k + kc]
        xtp = xtp.astype(bf)
        
        # a3 packed (DoubleRow fp8): quarter q at partition 32q, pair dim = a0/a1
        a3 = np.zeros((98, 2, TQ * BC), f32)
        for t in range(T):
            at = a0[sl] if t == 0 else fut_traj[t - 1, sl, 2:4]   # (BC, 2)
            qq, tq = t // TQ, t % TQ
            a3[32 * qq, :, tq * BC:(tq + 1) * BC] = at.T
        a3 = a3.astype(f8)
        in_maps.append({
            "xtp": xtp,
            "wbp": wbp,
            "wpk": wpk,
            "a3p": a3,
            "kap": kap,
            "bia": bia,
        })
    return in_maps


def unpack_outputs(results):
    mus = np.empty((T, B, 2), np.float32)
    stds = np.empty((T, B, 2), np.float32)
    for c in range(NCORES):
        sl = slice(c * BC, (c + 1) * BC)
        omu = results[c]["omu"].reshape(2, T, BC)
        ostd = results[c]["ostd"].reshape(2, T, BC)
        mus[:, sl, 0] = omu[0]
        mus[:, sl, 1] = omu[1]
        stds[:, sl, 0] = ostd[0]
        stds[:, sl, 1] = ostd[1]
    return mus, stds


def kernel(last_obs_state, enc_h_feat, z, sg, fut_traj,
           W_dh, b_dh, W_vel, b_vel, W_ih, b_ih, W_hh, b_hh,
           W_mu, b_mu, W_std, b_std):
    args = dict(
        last_obs_state=np.asarray(last_obs_state, np.float32),
        enc_h_feat=np.asarray(enc_h_feat, np.float32),
        z=np.asarray(z, np.float32),
        sg=np.asarray(sg, np.float32),
        fut_traj=np.asarray(fut_traj, np.float32),
        W_dh=np.asarray(W_dh, np.float32), b_dh=np.asarray(b_dh, np.float32),
        W_vel=np.asarray(W_vel, np.float32), b_vel=np.asarray(b_vel, np.float32),
        W_ih=np.asarray(W_ih, np.float32), b_ih=np.asarray(b_ih, np.float32),
        W_hh=np.asarray(W_hh, np.float32), b_hh=np.asarray(b_hh, np.float32),
        W_mu=np.asarray(W_mu, np.float32), b_mu=np.asarray(b_mu, np.float32),
        W_std=np.asarray(W_std, np.float32), b_std=np.asarray(b_std, np.float32),
    )
    nc = _get_nc()
    in_maps = make_in_maps(**args)
    res = run_bass_kernel_spmd(nc, in_maps, core_ids=list(range(NCORES)))
    return unpack_outputs(res.results)


# revision 45
# speedup vs baseline: 1.6025x; 1.0003x over previous
"""Trainium2 Bass/Tile kernel for nn_Decoder (GRU decoder with teacher forcing).

Math (per reference):
  zx  = [enc_h_feat, z]                    (B, 1056)
  h0  = zx @ W_dh.T + b_dh                 (B, 128)
  a0  = last_obs @ W_vel.T + b_vel         (B, 2)
  rel = (sg - last_obs[:, :2]) / dt        (B, 2)
  a_t = a0 if t==0 else fut_traj[t-1,:,2:4]
  x_t = [zx, a_t, rel]  -> GRUCell(x_t, h) -> mu_t, std_t

Device strategy (8 cores, batch-sharded, 2048 rows/core):
  - Feature-on-partition, batch-on-free layout; free chunks of 512.
  - Everything bf16 except PSUM (fp32) and final outputs (fp32).
  - Setup: [gi_r|gi_z|gi_n|h0](512 rows) = W_big.T @ XT with K=1065
    host-packed rows [zxT; sgT; loT; ones].  rel, b_ih, b_dh AND b_hh_{r,z}
    are folded into W_big / its ones-row on the host.
  - a-contributions: a3 for ALL steps resident in SBUF ([98, 6*BC] tile,
    quarter q of the steps on partitions {32q, 32q+1}) -> zero per-step DMAs;
    small-K matmuls with explicit tile_position.
  - Per step (bf16 matmuls, fp32 PSUM):
      psum_rz  = Whh_{r,z} @ h + Ka_{r,z} @ a_t + I @ gi_{r,z}
      rz       = sigmoid(psum_rz)                      [ScalarE]
      q        = (psum_hn + b_hh_n) * r                [DVE stt]
      psum_gin = I@gi_n + Ka_n @ a_t + I@q
      n        = tanh(psum_gin)                        [ScalarE]
      d = h - n [DVE 2x], e = z*d [GPSIMD], h' = n + e [DVE 2x]
      head: scattered-column lhsT accumulates mu/std pre-acts for ALL steps
      into 4 persistent PSUM tiles.
  - End: mu/std finals read the head PSUM directly (Identity / Exp(0.5x)).
  - DMAs: host packs xt/weights into contiguous [128, X] blobs; ~10 large
    input DMAs, all on the SP (sync) queue; nothing on the Pool queue.
Host does only sharding/transposes/weight packing (a0 is a (B,6)@(6,2)
matmul on host, ~0.4 MFLOP, negligible).
"""

import numpy as np
import ml_dtypes

import concourse.bass as bass
import concourse.mybir as mybir
import concourse.tile as tile
from concourse import bacc
from concourse.bass_utils import run_bass_kernel_spmd

F32 = mybir.dt.float32
F32R = mybir.dt.float32r
BF16 = mybir.dt.bfloat16
FP8 = mybir.dt.float8e4
DR = mybir.MatmulPerfMode.DoubleRow
AF = mybir.ActivationFunctionType
OP = mybir.AluOpType

B, T, MLP, ZD, H, NS, NP = 16384, 24, 1024, 32, 128, 6, 2
NCORES = 8
BC = B // NCORES            # 2048 rows per core
F = 512                     # free-dim chunk
NF = BC // F                # 4 chunks
KIN = MLP + ZD + NP + NS + 1  # 1065 = zx(1056) + sg(2) + lo(6) + ones(1)
NKC = (KIN + 127) // 128    # 9 K-chunks (8x128 + 41)
DT_CONST = 0.4 * 12
TQ = T // 4                 # steps per a3 partition-quarter (6)

# head accumulator rows: [mu0 xT | mu1 xT | pad | std0 xT | std1 xT]
STD_OFF = ((2 * T + 31) // 32) * 32   # 64
M_HEAD = STD_OFF + 2 * T              # 112


def build_nc(debug=False):
    nc = bacc.Bacc("TRN2", target_bir_lowering=False, debug=debug)

    # ---- DRAM I/O (all bf16 except biases / outputs) ----
    # xt packed: K-chunk k occupies cols [k*BC, (k+1)*BC) on partitions 0..kc
    xt_d = nc.dram_tensor("xtp", [128, NKC * BC], BF16, kind="ExternalInput").ap()
    # wbig packed: block (k, m) at cols [(k*4+m)*128, ...+128)
    wb_d = nc.dram_tensor("wbp", [128, NKC * 4 * 128], BF16, kind="ExternalInput").ap()
    # misc weights packed: [whht(384) | wmsx(T*112) | ident(128)]
    WPCK = 3 * H + T * M_HEAD + H
    wp_d = nc.dram_tensor("wpk", [128, WPCK], F32, kind="ExternalInput").ap()
    # a3: quarter q at partitions {32q, 32q+1}; cols (t%TQ)*BC + j
    a3_d = nc.dram_tensor("a3p", [98, 2, TQ * BC], FP8, kind="ExternalInput").ap()
    # ka: same partition layout; cols g*128.. for gate g in (r, z, n)
    ka_d = nc.dram_tensor("kap", [98, 2, 3 * H], FP8, kind="ExternalInput").ap()
    # biases fp32: col0 = b_hh_n (128 rows); col1 = bmu (48); col2 = 0.5*bstd
    bia_d = nc.dram_tensor("bia", [128, 3], F32, kind="ExternalInput").ap()
    omu_d = nc.dram_tensor("omu", [2 * T, BC], F32, kind="ExternalOutput").ap()
    ostd_d = nc.dram_tensor("ostd", [2 * T, BC], F32, kind="ExternalOutput").ap()

    with tile.TileContext(nc) as tc:
        with tc.tile_pool(name="persist", bufs=1) as pp:
            gi_r = pp.tile([H, BC], F32R)
            gi_z = pp.tile([H, BC], F32R)
            gi_n = pp.tile([H, BC], F32R)
            hA = pp.tile([H, BC], F32R)
            hB = pp.tile([H, BC], F32R)
            wpk = pp.tile([128, WPCK], F32R)
            a3 = pp.tile([98, 2, TQ * BC], FP8)
            ka = pp.tile([98, 2, 3 * H], FP8)
            bia = pp.tile([128, 3], F32)
            mu_sb = pp.tile([2 * T, BC], F32)
            std_sb = pp.tile([2 * T, BC], F32)

            whht = wpk[:, 0:3 * H]
            wmsx = wpk[:, 3 * H:3 * H + T * M_HEAD]
            ident = wpk[:, 3 * H + T * M_HEAD:WPCK]
            bhhn = bia[:, 0:1]
            bmu = bia[0:2 * T, 1:2]
            bstd = bia[0:2 * T, 2:3]


            gi_dst = [gi_r, gi_z, gi_n, hA]

            # ---- setup: [gi | h0] = W_big.T @ XT  (bf16) ----
            with tc.tile_pool(name="xtp", bufs=1) as xtp, \
                 tc.tile_pool(name="wbp", bufs=1) as wbp, \
                 tc.tile_pool(name="sps", bufs=8, space="PSUM") as sps:
                wb = wbp.tile([128, NKC * 4 * 128], BF16, name="wb", tag="wb")
                xt = xtp.tile([128, NKC * BC], BF16, name="xt", tag="xt")
                # interleaved per-k DMAs: first matmuls start after one pair
                for k in range(NKC):
                    nc.sync.dma_start(wb[:, k * 512:(k + 1) * 512],
                                      wb_d[:, k * 512:(k + 1) * 512])
                    nc.sync.dma_start(xt[:, k * BC:(k + 1) * BC],
                                      xt_d[:, k * BC:(k + 1) * BC])
                # loop-phase inputs (needed only after setup finishes)
                nc.sync.dma_start(wpk[:], wp_d.bitcast(F32R))
                nc.sync.dma_start(ka[:], ka_d)
                nc.sync.dma_start(bia[:], bia_d)
                nc.sync.dma_start(a3[:], a3_d)

                for m in range(4):
                    for c in range(NF):
                        ps = sps.tile([128, F], F32, name="setps", tag="setps")
                        for k in range(NKC):
                            kc = min(128, KIN - 128 * k)
                            nc.tensor.matmul(
                                ps[:],
                                wb[0:kc, (k * 4 + m) * 128:(k * 4 + m) * 128 + 128],
                                xt[0:kc, k * BC + c * F:k * BC + c * F + F],
                                start=(k == 0), stop=(k == NKC - 1),
                            )
                        nc.vector.tensor_copy(gi_dst[m][:, c * F:(c + 1) * F], ps[:])

            # ---- recurrence (software-pipelined issue order) ----
            # Per (t, c) block the PE issues: head(t-1, c), then the 10 loop
            # matmuls for (t, c).  The elementwise tail for (t, c) is issued
            # one block later (deferred) so no engine queue head-of-line
            # blocks on a cross-engine dependency.
            with tc.tile_pool(name="gp", bufs=6) as gp, \
                 tc.tile_pool(name="ps", bufs=4, space="PSUM") as psp, \
                 tc.tile_pool(name="phd", bufs=1, space="PSUM") as phd:
                psum_hd = [
                    phd.tile([M_HEAD, F], F32, name=f"pshd{c}", tag=f"pshd{c}")
                    for c in range(NF)
                ]
                from collections import deque
                pend = None   # (t, c, ps_gin, gr, gz, q) from the previous block
                hq = deque()  # (t, c, nt, e) awaiting the h' add, issued 3 blocks late

                def issue_hprime():
                    pt, pc, nt, e = hq.popleft()
                    pn = hB if pt % 2 == 0 else hA
                    pcs = slice(pc * F, (pc + 1) * F)
                    nc.vector.tensor_tensor(pn[:, pcs], nt[:], e[:], op=OP.add)

                def flush_npre():
                    # DVE: npre for the previous block, ahead of this block's q
                    if pend is None:
                        return None
                    pt, pc, ps_gin, gr, gz, q = pend
                    npre = gp.tile([128, F], F32, name="np", tag="np")
                    nc.vector.tensor_tensor(npre[:], ps_gin[:], q[:], op=OP.add)
                    return npre

                def flush_tail(npre):
                    # Act: tanh (queued after this block's sigmoids);
                    # Pool: d, e.  h' enqueues for a later block.
                    nonlocal pend
                    if pend is None:
                        return
                    pt, pc, ps_gin, gr, gz, q = pend
                    pend = None
                    pcs = slice(pc * F, (pc + 1) * F)
                    ph = hA if pt % 2 == 0 else hB
                    nt = gp.tile([128, F], F32, name="nt", tag="nt")
                    nc.scalar.activation(nt[:], npre[:], AF.Tanh)
                    d = gp.tile([128, F], F32, name="d", tag="d")
                    nc.gpsimd.tensor_tensor(d[:], ph[:, pcs], nt[:], op=OP.subtract)
                    e = gp.tile([128, F], F32, name="e", tag="e")
                    nc.gpsimd.tensor_tensor(e[:], gz[:], d[:], op=OP.mult)
                    hq.append((pt, pc, nt, e))

                for t in range(T):
                    hcur = hA if t % 2 == 0 else hB
                    hnxt = hB if t % 2 == 0 else hA
                    hprv = hnxt  # h produced at t-1 lives in the other buffer
                    ar = 32 * (t // TQ)          # a3 partition base for this step
                    ac = (t % TQ) * BC           # a3 col base
                    for c in range(NF):
                        cs = slice(c * F, (c + 1) * F)
                        hs = hcur[:, cs]
                        a_sl = a3[ar:ar + 1, :, ac + c * F:ac + c * F + F]
                        if t > 0:
                            # head for (t-1, c): reads hcur (h of step t-1)
                            nc.tensor.matmul(
                                psum_hd[c][:],
                                wmsx[:, (t - 1) * M_HEAD:t * M_HEAD],
                                hs,
                                start=(t - 1 == 0), stop=False,
                                skip_group_check=True,
                            )
                        ps_r = psp.tile([128, F], F32, name="psr", tag="ps")
                        nc.tensor.matmul(ps_r[:], whht[:, 0:H], hs,
                                         start=True, stop=False)
                        nc.tensor.matmul(ps_r[:], ka[ar:ar + 1, :, 0:H], a_sl,
                                         start=False, stop=False, perf_mode=DR,
                                         tile_position=(ar, 0))
                        nc.tensor.matmul(ps_r[:], ident, gi_r[:, cs],
                                         start=False, stop=True)
                        ps_z = psp.tile([128, F], F32, name="psz", tag="ps")
                        nc.tensor.matmul(ps_z[:], whht[:, H:2 * H], hs,
                                         start=True, stop=False)
                        nc.tensor.matmul(ps_z[:], ka[ar:ar + 1, :, H:2 * H], a_sl,
                                         start=False, stop=False, perf_mode=DR,
                                         tile_position=(ar, 0))
                        nc.tensor.matmul(ps_z[:], ident, gi_z[:, cs],
                                         start=False, stop=True)
                        ps_hn = psp.tile([128, F], F32, name="psh", tag="ps")
                        nc.tensor.matmul(ps_hn[:], whht[:, 2 * H:3 * H], hs,
                                         start=True, stop=True)
                        ps_gin = psp.tile([128, F], F32, name="psg", tag="ps")
                        nc.tensor.matmul(ps_gin[:], ident, gi_n[:, cs],
                                         start=True, stop=False)
                        nc.tensor.matmul(ps_gin[:], ka[ar:ar + 1, :, 2 * H:3 * H],
                                         a_sl, start=False, stop=True, perf_mode=DR,
                                         tile_position=(ar, 0))
                        if len(hq) >= 2:
                            issue_hprime()
                        npre_prev = flush_npre()
                        gr = gp.tile([128, F], F32, name="gr", tag="gr")
                        nc.scalar.activation(gr[:], ps_r[:], AF.Sigmoid)
                        gz = gp.tile([128, F], F32, name="gz", tag="gz")
                        nc.scalar.activation(gz[:], ps_z[:], AF.Sigmoid)
                        q = gp.tile([128, F], F32, name="q", tag="q")
                        nc.vector.scalar_tensor_tensor(
                            q[:], ps_hn[:], bhhn, gr[:],
                            op0=OP.add, op1=OP.mult,
                        )
                        flush_tail(npre_prev)
                        pend = (t, c, ps_gin, gr, gz, q)
                npre_prev = flush_npre()
                flush_tail(npre_prev)
                while hq:
                    issue_hprime()
                # post-loop heads for t = T-1 (reads h of the last step)
                hlast = hB if (T - 1) % 2 == 0 else hA
                for c in range(NF):
                    nc.tensor.matmul(
                        psum_hd[c][:],
                        wmsx[:, (T - 1) * M_HEAD:T * M_HEAD],
                        hlast[:, c * F:(c + 1) * F],
                        start=False, stop=True,
                        skip_group_check=True,
                    )

                # ---- finals straight from head PSUM ----
                for c in range(NF):
                    cs = slice(c * F, (c + 1) * F)
                    nc.scalar.activation(mu_sb[:, cs], psum_hd[c][0:2 * T, :],
                                         AF.Identity, bias=bmu)
                nc.sync.dma_start(omu_d, mu_sb[:])
                for c in range(NF):
                    cs = slice(c * F, (c + 1) * F)
                    nc.scalar.activation(std_sb[:, cs],
                                         psum_hd[c][STD_OFF:STD_OFF + 2 * T, :],
                                         AF.Exp, bias=bstd, scale=0.5)
            nc.sync.dma_start(ostd_d, std_sb[:])

    nc.compile()
    return nc


_NC_CACHE = {}


def _get_nc(debug=False):
    if "nc" not in _NC_CACHE:
        _NC_CACHE["nc"] = build_nc(debug=debug)
    return _NC_CACHE["nc"]


def make_in_maps(last_obs_state, enc_h_feat, z, sg, fut_traj,
                 W_dh, b_dh, W_vel, b_vel, W_ih, b_ih, W_hh, b_hh,
                 W_mu, b_mu, W_std, b_std):
    f32 = np.float32
    bf = ml_dtypes.bfloat16
    f8 = ml_dtypes.float8_e4m3

    # ---- weight packing (core-independent) ----
    # W_big: (KIN, 512) ; out cols = [gi_r, gi_z, gi_n, h0]
    wbig = np.zeros((KIN, 512), f32)
    wbig[0:1056, 0:384] = W_ih[:, 0:1056].T
    wbig[0:1056, 384:512] = W_dh.T
    # sg rows: rel = (sg - lo[:, :2])/dt feeds W_ih[:, 1058:1060]
    wbig[1056:1058, 0:384] = (W_ih[:, 1058:1060] / DT_CONST).T
    # lo rows (6): first two carry -W_rel/dt
    wbig[1058:1060, 0:384] = (-W_ih[:, 1058:1060] / DT_CONST).T
    # ones row: input-side biases; b_hh_{r,z} folded in here too
    wbig[1064, 0:384] = b_ih
    wbig[1064, 0:256] += b_hh[0:256]
    wbig[1064, 384:512] = b_dh
    # packed layout [128, NKC*4*128]: block (k, m) at cols (k*4+m)*128
    wbp = np.zeros((128, NKC * 4 * 128), f32)
    for k in range(NKC):
        kc = min(128, KIN - 128 * k)
        for m in range(4):
            wbp[0:kc, (k * 4 + m) * 128:(k * 4 + m) * 128 + 128] = \
                wbig[128 * k:128 * k + kc, 128 * m:128 * (m + 1)]
    wbp = wbp.astype(bf)
    

    # misc weights packed: [whht | wmsx | ident]
    whht = np.ascontiguousarray(W_hh.T)                     # (128, 384)
    wmsx = np.zeros((H, T, M_HEAD), f32)
    for t in range(T):
        wmsx[:, t, t] = W_mu[0]
        wmsx[:, t, T + t] = W_mu[1]
        wmsx[:, t, STD_OFF + t] = W_std[0]
        wmsx[:, t, STD_OFF + T + t] = W_std[1]
    wpk = np.concatenate(
        [whht, wmsx.reshape(H, T * M_HEAD), np.eye(H, dtype=f32)], axis=1
    ).astype(f32)

    # ka: Wa pairs for each quarter at partition 32q (DoubleRow fp8 layout)
    kap = np.zeros((98, 2, 3 * H), f32)
    for qq in range(4):
        for i in range(2):
            kap[32 * qq, i, 0:H] = W_ih[0:128, 1056 + i]
            kap[32 * qq, i, H:2 * H] = W_ih[128:256, 1056 + i]
            kap[32 * qq, i, 2 * H:3 * H] = W_ih[256:384, 1056 + i]
    kap = kap.astype(f8)

    bia = np.zeros((128, 3), f32)
    bia[:, 0] = b_hh[256:384]
    bia[0:2 * T, 1] = np.repeat(b_mu, T)
    bia[0:2 * T, 2] = 0.5 * np.repeat(b_std, T)

    # host-side tiny matmul for a0 (0.4 MFLOP)
    a0 = last_obs_state @ W_vel.T + b_vel                    # (B, 2)

    in_maps = []
    for cidx in range(NCORES):
        sl = slice(cidx * BC, (cidx + 1) * BC)
        xt = np.empty((KIN, BC), f32)
        xt[0:MLP] = enc_h_feat[sl].T
        xt[MLP:1056] = z[sl].T
        xt[1056:1058] = sg[sl].T
        xt[1058:1064] = last_obs_state[sl].T
        xt[1064] = 1.0
        xtp = np.zeros((128, NKC * BC), f32)
        for k in range(NKC):
            kc = min(128, KIN - 128 * k)
            xtp[0:kc, k * BC:k * BC + BC] = xt[128 * k:128 * k + kc]
        xtp = xtp.astype(bf)
        
        # a3 packed (DoubleRow fp8): quarter q at partition 32q, pair dim = a0/a1
        a3 = np.zeros((98, 2, TQ * BC), f32)
        for t in range(T):
            at = a0[sl] if t == 0 else fut_traj[t - 1, sl, 2:4]   # (BC, 2)
            qq, tq = t // TQ, t % TQ
            a3[32 * qq, :, tq * BC:(tq + 1) * BC] = at.T
        a3 = a3.astype(f8)
        in_maps.append({
            "xtp": xtp,
            "wbp": wbp,
            "wpk": wpk,
            "a3p": a3,
            "kap": kap,
            "bia": bia,
        })
    return in_maps


def unpack_outputs(results):
    mus = np.empty((T, B, 2), np.float32)
    stds = np.empty((T, B, 2), np.float32)
    for c in range(NCORES):
        sl = slice(c * BC, (c + 1) * BC)
        omu = results[c]["omu"].reshape(2, T, BC)
        ostd = results[c]["ostd"].reshape(2, T, BC)
        mus[:, sl, 0] = omu[0]
        mus[:, sl, 1] = omu[1]
        stds[:, sl, 0] = ostd[0]
        stds[:, sl, 1] = ostd[1]
    return mus, stds


def kernel(last_obs_state, enc_h_feat, z, sg, fut_traj,
           W_dh, b_dh, W_vel, b_vel, W_ih, b_ih, W_hh, b_hh,
           W_mu, b_mu, W_std, b_std):
    args = dict(
        last_obs_state=np.asarray(last_obs_state, np.float32),
        enc_h_feat=np.asarray(enc_h_feat, np.float32),
        z=np.asarray(z, np.float32),
        sg=np.asarray(sg, np.float32),
        fut_traj=np.asarray(fut_traj, np.float32),
        W_dh=np.asarray(W_dh, np.float32), b_dh=np.asarray(b_dh, np.float32),
        W_vel=np.asarray(W_vel, np.float32), b_vel=np.asarray(b_vel, np.float32),
        W_ih=np.asarray(W_ih, np.float32), b_ih=np.asarray(b_ih, np.float32),
        W_hh=np.asarray(W_hh, np.float32), b_hh=np.asarray(b_hh, np.float32),
        W_mu=np.asarray(W_mu, np.float32), b_mu=np.asarray(b_mu, np.float32),
        W_std=np.asarray(W_std, np.float32), b_std=np.asarray(b_std, np.float32),
    )
    nc = _get_nc()
    in_maps = make_in_maps(**args)
    res = run_bass_kernel_spmd(nc, in_maps, core_ids=list(range(NCORES)))
    return unpack_outputs(res.results)
